# Initial kernel scaffold
#
"""Multi-head 3D attention (8 heads, C=512, N=16^3=4096) on 8 Trainium2 cores.

Sharding: one head per NeuronCore (head-parallel). Each core receives the
full token activations plus its head's slice of the qkv/out projection
weights, computes its head's attention and its partial contribution to the
output projection; the host sums the 8 fp16 partial outputs in fp32.

Per-core algorithm (S^T orientation -> no transposes anywhere):
  xT   = x.reshape(C, N)                   # [512, 4096] fp16, channel-major
  q/k  = W_{qk} @ xT in ONE matmul group   # [128, 512] psum: q rows 0:64,
                                           #   k rows 64:128 (W_k pre-scaled
                                           #   by A = 1024*log2(e) on host)
  v    = xT.T @ Wv.T                       # [4096, 64] bf16 (keys on parts)
  S^T  = kT-tile.T @ qT                    # 2x [128 keys, 512 q] PSUM = A*s
  P^T  = softmax numerator, column-split across 2 engines per key tile:
           one 512-query half -> ACT:  exp(A*s * 8/A + delta)      (exact)
           other half         -> DVE:  int16(A*s + B) bitcast bf16
                                       (Schraudolph exp, 1 tensor_scalar op)
         halves alternate by kt so every query sees a 50/50 mix; per-tile
         softmax latency ~0.7us, and neither engine exceeds ~60% load
  o_aug= [v, 1].T @ P^T                    # [65, 1024] PSUM; row 64 = denom
  o    = o_aug[:64] * (1/denom)            # reciprocal_approx_fast +
                                           #   gpsimd broadcast + DVE mul
  outp = w_out_h @ o                       # [512, 4096] fp16 partials

Softmax numerics: the Schraudolph bit-trick writes round(A*s + B) as int16
whose bits ARE the bf16 exp(8s+delta): A*s = 128*log2(e)*8s, and
B = 128*(127 + c) + delta*128*log2(e) with c = -0.0427 centering the
piecewise-linear-mantissa error (+-3%) around 1 (HW converts fp32->int16
with round-to-nearest; verified by probe). delta = -3.5 shifts all logits
uniformly (softmax-invariant) to center the observed logit range
[-82.6, 88.1] inside the int16-safe window (-88.0, +88.7); it also pulls
the peak numerator well below fp32-overflow in the o accumulation and the
peak denominator below reciprocal_approx_fast's undefined |x|>~1e38 zone.
Measured end-to-end rel err: 7.5e-3 (gate 2e-2).

Pipeline/schedule (the PE is the bottleneck: S + P@V stream 109 us of
columns at 1 cycle/row fp16, projections ~20 us):
 - x streams in over 3 DMA queues (sync/gpsimd/scalar); q/k/v projections
   are JIT'd inside query group 0 so they hide under the DMA shadow.
 - per key tile the PE issues S_A, S_B then the o-matmuls of kt-3: the
   trail-3 gives the cross-engine softmax chain (sem + 0.7us exp + sem,
   ~1.5us) a ~2.6us window, so the PE never waits on softmax.
 - PSUM: two 3-buf pools of [128,512] S tiles (one per query half, 6
   banks) + single-buffered [65,1024] o psum (2 banks) = 8 banks. The o
   psum is freed ~1.2us after a group ends by a fast unnormalized flush;
   normalization runs in place on SBUF off the critical path (the last
   group instead uses a 512-wide pipelined chain fused into the flush).
 - each group's output projection + fp16 conversion + DMA is interleaved
   into the NEXT group's attention; only qg3's runs as a ~10us tail. Each
   group's first 3 S tiles are pre-emitted in the previous group's tail so
   the o pipeline restarts hot across group boundaries.
 - discretionary PE work is minimized (4 warm-up fillers only): sustained
   PE activity triggers the chip's HAM duty-cycle throttle (k=8 -> k=4 at
   ~75% avg util), which is also the main source of run-to-run variance
   (~195us unthrottled, ~218us when a 50us half-speed window lands
   mid-run). Filler matmuls to fight clock-gating are counterproductive.

Custom-DVE gotcha (HW-verified): InstCustomDveAnt ignores the input AP's
partition offset -- reciprocal_approx_fast on ops[64:65,:] silently read
partition 0. The denominator row is first copied to a partition-0 SBUF
tile with a plain tensor_copy (which handles offsets correctly).
"""

import sys

for _p in ("/opt/trn_rl_repo",):
    if _p not in sys.path:
        sys.path.insert(0, _p)

import math

import numpy as np

C = 512          # channels
N = 4096         # tokens (16*16*16)
HEADS = 8
DH = C // HEADS  # 64
NCORES = 8

KT = 128                 # key-tile size (S^T partition dim)
NKT = N // KT            # 32
QG = 1024                # queries per o-psum accumulation group
NQG = N // QG            # 4
SW = 1024                # S-tile width (queries per exp call)
MV = 512                 # max matmul free dim (one PSUM bank)

A_SCALE = 1024.0 * math.log2(math.e)     # folded into W_k on host
DELTA = -3.5                              # uniform logit shift
C_CORR = -0.0427                          # Schraudolph centering
B_DVE = 128.0 * (127.0 + C_CORR) + DELTA * 128.0 * math.log2(math.e)
EXP_SCALE = 8.0 / A_SCALE

_compiled = None


def _build():
    import concourse.tile as tile
    from concourse import bacc, mybir

    F32 = mybir.dt.float32
    F16 = mybir.dt.float16
    BF16 = mybir.dt.bfloat16
    I16 = mybir.dt.int16
    EXP = mybir.ActivationFunctionType.Exp
    MUL = mybir.AluOpType.mult
    ADD = mybir.AluOpType.add
    NCT = C // 128  # 4 channel tiles

    import os
    kdebug = bool(int(os.environ.get("KDEBUG", "0")))

    nc = bacc.Bacc("TRN2", num_devices=NCORES)
    xT_d = nc.dram_tensor("xT", [C, N], F16, kind="ExternalInput")
    # columns 0:64 = Wq^T, 64:128 = A*Wk^T, 128:192 = Wv^T (this head's rows)
    wqkvT_d = nc.dram_tensor("wqkvT", [C, 3 * DH], F16, kind="ExternalInput")
    # w_out[:, head_cols].T  -> [64, 512]
    w_outT_d = nc.dram_tensor("w_outT", [DH, C], BF16, kind="ExternalInput")
    outp_d = nc.dram_tensor("outp", [C, N], F16, kind="ExternalOutput")
    if kdebug:
        dbg_qT = nc.dram_tensor("dbg_qT", [DH, N], F16, kind="ExternalOutput")
        dbg_kT = nc.dram_tensor("dbg_kT", [DH, N], F16, kind="ExternalOutput")
        dbg_vaug = nc.dram_tensor("dbg_vaug", [128, NKT * (DH + 1)], BF16,
                                  kind="ExternalOutput")
        dbg_osb = nc.dram_tensor("dbg_osb", [DH, N], BF16,
                                 kind="ExternalOutput")
        dbg_recip = nc.dram_tensor("dbg_recip", [1, N], F32,
                                   kind="ExternalOutput")
        dbg_recipb = nc.dram_tensor("dbg_recipb", [DH, N], F32,
                                    kind="ExternalOutput")
        dbg_p = nc.dram_tensor("dbg_p", [128, NKT * SW], BF16,
                               kind="ExternalOutput")

    with tile.TileContext(nc) as tc:
        with tc.tile_pool(name="const", bufs=1) as const:
            # ---- persistent SBUF tensors ----
            xt = [const.tile([128, N], F16, tag=f"x{i}", name=f"x{i}")
                  for i in range(NCT)]
            wqkv = [const.tile([128, 3 * DH], F16, tag=f"w{i}", name=f"w{i}")
                    for i in range(NCT)]
            woutT = const.tile([DH, C], BF16, tag="wo")
            qT = const.tile([DH, N], F16, tag="qT")
            kT = const.tile([DH, N], F16, tag="kT")
            vaug = const.tile([128, NKT, DH + 1], BF16, tag="vaug")
            o_sb = const.tile([DH, N], BF16, tag="o")        # o^T normalized
            den = const.tile([1, N], F32, tag="den")         # softmax denom
            recip = const.tile([1, N], F32, tag="recip")     # 1/denominator
            recipb = const.tile([DH, N], F32, tag="recipb")  # bcast to 64p
            # P^T tiles for one full query group (decouples P@v from exp)
            pstore = const.tile([128, NKT, SW], BF16, tag="pstore")

            # ones column of vaug (o-matmul denominator row), written once
            nc.gpsimd.memset(vaug[:, :, DH:DH + 1], 1.0)
            # per-partition bias AP for the ACT exp (delta logit shift)
            dbias = const.tile([128, 1], F32, tag="dbias")
            nc.vector.memset(dbias, DELTA)

            # inputs across three DMA queues so the ramp-critical pieces land
            # in parallel: slice0 (gates qk0) on sync, wqkv + slice2 on
            # scalar, slice1 on gpsimd; woutT is not needed until qg1
            dma_engs = (nc.sync, nc.gpsimd, nc.scalar, nc.sync)
            for i in range(NCT):
                nc.sync.dma_start(
                    out=wqkv[i], in_=wqkvT_d.ap()[i * 128:(i + 1) * 128, :])
            nc.sync.dma_start(out=woutT, in_=w_outT_d.ap())
            for si, (lo, hi) in enumerate(
                    ((0, 512), (512, 1024), (1024, 2048), (2048, N))):
                for i in range(NCT):
                    dma_engs[si].dma_start(
                        out=xt[i][:, lo:hi],
                        in_=xT_d.ap()[i * 128:(i + 1) * 128, lo:hi])

            def qk_chunk(pool, ch):
                """q AND k projection for token chunk ch in one matmul group:
                psum rows 0:64 = q, 64:128 = A*k (host-folded scale)."""
                sl = slice(ch * MV, (ch + 1) * MV)
                ps = pool.tile([128, MV], F32, tag="s", name=f"psqk{ch}")
                for ct in range(NCT):
                    nc.tensor.matmul(ps, lhsT=wqkv[ct][:, 0:2 * DH],
                                     rhs=xt[ct][:, sl],
                                     start=(ct == 0), stop=(ct == NCT - 1))
                nc.vector.tensor_copy(out=qT[:, sl], in_=ps[0:DH, :])
                nc.vector.tensor_copy(out=kT[:, sl], in_=ps[DH:2 * DH, :])

            def v_tile(pool, kt_i):
                """v projection for key tile kt_i -> vaug[:, kt_i, 0:64]."""
                ps = pool.tile([128, MV], F32, tag="s", name=f"psv{kt_i}")
                for ct in range(NCT):
                    nc.tensor.matmul(ps[:, 0:DH],
                                     lhsT=xt[ct][:, kt_i * KT:(kt_i + 1) * KT],
                                     rhs=wqkv[ct][:, 2 * DH:3 * DH],
                                     start=(ct == 0), stop=(ct == NCT - 1))
                nc.scalar.copy(out=vaug[:, kt_i, 0:DH], in_=ps[:, 0:DH])

            # ---- attention (projections JIT'd into query group 0) ----
            # PSUM: s_ps 6 x [128,512] (6 banks) + ops [65,1024] (2 banks,
            # single-buffered with fast flush) = 8 banks exactly.
            with tc.tile_pool(name="s_psA", bufs=3, space="PSUM") as s_psA, \
                 tc.tile_pool(name="s_psB", bufs=3, space="PSUM") as s_psB, \
                 tc.tile_pool(name="o_ps", bufs=1, space="PSUM") as o_ps, \
                 tc.tile_pool(name="out_sb", bufs=3) as out_sb:
                s_ps = s_psA
                # warm-up fillers need only the FIRST two weight DMAs; the
                # PE enters the projections already at full clock. Kept
                # minimal: PE activity feeds the chip's HAM throttle budget,
                # so every discretionary matmul eventually costs real time.
                last_filler = None
                for wf in range(4):
                    last_filler = s_ps.tile([128, MV], F32, tag="s",
                                            name=f"warm{wf}")
                    nc.tensor.matmul(last_filler[:, 0:192],
                                     lhsT=wqkv[wf % 2][:, 0:128],
                                     rhs=wqkv[(wf + 1) % 2][:, :],
                                     start=True, stop=True,
                                     skip_group_check=True)
                fzt = const.tile([128, 1], F32, tag="fzt")
                nc.vector.tensor_copy(out=fzt, in_=last_filler[:, 0:1])

                def out_block(qgp, ct):
                    """output projection for query group qgp, channel block
                    ct: 2 matmuls -> 2 psum tiles, parallel ACT+DVE fp16
                    copies, one DMA. Interleaved into the NEXT query group's
                    attention so only qg3's blocks run as a tail."""
                    base = qgp * QG
                    pso = []
                    for mv in range(2):
                        p = s_ps.tile([128, MV], F32, tag="s",
                                      name=f"pso{qgp}_{ct}_{mv}")
                        nc.tensor.matmul(
                            p, lhsT=woutT[:, ct * 128:(ct + 1) * 128],
                            rhs=o_sb[:, base + mv * MV:base + (mv + 1) * MV],
                            start=True, stop=True)
                        pso.append(p)
                    ot = out_sb.tile([128, 1024], F16, tag="ot",
                                     name=f"ot{qgp}_{ct}")
                    nc.scalar.copy(out=ot[:, 0:MV], in_=pso[0])
                    nc.vector.tensor_copy(out=ot[:, MV:2 * MV], in_=pso[1])
                    if qgp == 0 and ct == 0:
                        # + 0 * filler keeps the warm-up matmuls alive
                        nc.vector.scalar_tensor_tensor(
                            out=ot[:, 0:1], in0=fzt, scalar=0.0,
                            in1=ot[:, 0:1], op0=MUL, op1=ADD)
                    dma_engs[ct % 3].dma_start(
                        out=outp_d.ap()[ct * 128:(ct + 1) * 128,
                                        base:base + QG],
                        in_=ot)

                # first two q/k chunks feed the first S matmuls; the rest of
                # the projections are emitted just-in-time inside qg 0
                qk_chunk(s_ps, 0)
                qk_chunk(s_ps, 1)

                def emit_s(qgx, ktx):
                    """S matmul pair + column-split softmax dispatch for key
                    tile ktx of query group qgx: one 1-bank psum tile per
                    512-query half so each engine starts as soon as ITS half
                    lands; ACT exps one half, DVE bit-tricks the other,
                    alternating by kt (50/50 exact/approx per query)."""
                    qx0 = qgx * QG
                    sps = [None, None]
                    for mv in range(SW // MV):
                        pool = s_psA if mv == 0 else s_psB
                        sps[mv] = pool.tile([128, MV], F32, tag="s",
                                            name=f"sps{qgx}_{ktx}_{mv}")
                        nc.tensor.matmul(
                            sps[mv],
                            lhsT=kT[:, ktx * KT:(ktx + 1) * KT],
                            rhs=qT[:, qx0 + mv * MV: qx0 + (mv + 1) * MV],
                            start=True, stop=True)
                    a = (ktx + 2 * qgx) % 2
                    ha = slice(a * MV, a * MV + MV)
                    hd = slice((1 - a) * MV, (1 - a) * MV + MV)
                    nc.scalar.activation(out=pstore[:, ktx, ha],
                                         in_=sps[a], func=EXP,
                                         scale=EXP_SCALE, bias=dbias)
                    pi16 = pstore[:, ktx, hd].bitcast(I16)
                    nc.vector.tensor_scalar(out=pi16, in0=sps[1 - a],
                                            scalar1=B_DVE,
                                            scalar2=None, op0=ADD)

                for qg in range(NQG):
                    q0 = qg * QG
                    ops = o_ps.tile([DH + 1, QG], F32, tag="ops",
                                    name=f"ops{qg}")
                    # groups > 0 had their first 3 S tiles pre-emitted in the
                    # previous group's tail, so the o pipeline starts hot
                    s_start = 0 if qg == 0 else 3
                    for kt_i in range(NKT + 3):
                        if qg == 0 and kt_i < NKT:
                            if kt_i % 4 == 0 and kt_i // 4 + 2 < N // MV:
                                qk_chunk(s_ps, kt_i // 4 + 2)
                            v_tile(s_ps, kt_i)
                        if qg >= 1 and kt_i in (6, 12, 18, 24):
                            out_block(qg - 1, (kt_i - 6) // 6)
                        if s_start <= kt_i < NKT:
                            emit_s(qg, kt_i)
                        if kt_i >= 3:
                            # o trails S by THREE tiles: the softmax chain
                            # (sem + exp + sem, ~1.5us) hides well under the
                            # PE work window, so the PE never stalls on it
                            ot_i = kt_i - 3
                            for mv in range(SW // MV):
                                nc.tensor.matmul(
                                    ops[:, mv * MV:(mv + 1) * MV],
                                    lhsT=vaug[:, ot_i, :],
                                    rhs=pstore[:, ot_i, mv * MV:(mv + 1) * MV],
                                    start=(ot_i == 0),
                                    stop=(ot_i == NKT - 1))
                    if qg < NQG - 1:
                        # fast flush (frees the single ops buffer in ~1.2us);
                        # normalization then runs in place on o_sb off-path,
                        # hidden under the next query group
                        sl = slice(q0, q0 + QG)
                        nc.vector.tensor_copy(out=o_sb[:, sl],
                                              in_=ops[0:DH, :])
                        nc.scalar.copy(out=den[:, sl], in_=ops[DH:DH + 1, :])
                        nc.vector.reciprocal_approx_fast(out=recip[:, sl],
                                                         in_=den[:, sl])
                        nc.gpsimd.partition_broadcast(recipb[:, sl],
                                                      recip[:, sl])
                        nc.vector.tensor_mul(o_sb[:, sl], o_sb[:, sl],
                                             recipb[:, sl])
                    else:
                        # last group: nothing hides this chain, so pipeline
                        # it in 512-wide steps across ACT/DVE/POOL with the
                        # normalize fused into the flush (~3.4us vs ~7us)
                        for h2 in range(QG // MV):
                            hps = slice(h2 * MV, (h2 + 1) * MV)
                            hsl = slice(q0 + h2 * MV, q0 + (h2 + 1) * MV)
                            nc.scalar.copy(out=den[:, hsl],
                                           in_=ops[DH:DH + 1, hps])
                            nc.vector.reciprocal_approx_fast(
                                out=recip[:, hsl], in_=den[:, hsl])
                            nc.gpsimd.partition_broadcast(recipb[:, hsl],
                                                          recip[:, hsl])
                            nc.vector.scalar_tensor_tensor(
                                out=o_sb[:, hsl], in0=ops[0:DH, hps],
                                scalar=1.0, in1=recipb[:, hsl],
                                op0=MUL, op1=MUL)

                    if qg + 1 < NQG:
                        # overlap the boundary: next group's first S tiles +
                        # softmax run under this group's trailing o-matmuls
                        for k2 in range(3):
                            emit_s(qg + 1, k2)

                # tail: last query group's output projection
                for ct in range(NCT):
                    out_block(NQG - 1, ct)

            if kdebug:
                nc.sync.dma_start(out=dbg_qT.ap(), in_=qT)
                nc.sync.dma_start(out=dbg_kT.ap(), in_=kT)
                nc.sync.dma_start(out=dbg_vaug.ap(), in_=vaug)
                nc.sync.dma_start(out=dbg_osb.ap(), in_=o_sb)
                nc.sync.dma_start(out=dbg_recip.ap(), in_=recip)
                nc.sync.dma_start(out=dbg_recipb.ap(), in_=recipb)
                nc.sync.dma_start(out=dbg_p.ap(), in_=pstore)

    nc.compile()
    return nc


def _get_compiled():
    global _compiled
    if _compiled is None:
        _compiled = _build()
    return _compiled


def make_in_maps(x, w_qkv, w_out):
    import ml_dtypes
    xT = np.ascontiguousarray(x.reshape(C, N).astype(np.float16))
    in_maps = []
    for h in range(NCORES):
        wq = w_qkv[h * DH:(h + 1) * DH, :]
        wk = w_qkv[C + h * DH:C + (h + 1) * DH, :] * np.float32(A_SCALE)
        wv = w_qkv[2 * C + h * DH:2 * C + (h + 1) * DH, :]
        wqkvT = np.ascontiguousarray(
            np.concatenate([wq, wk, wv], axis=0).T.astype(np.float16))
        w_outT = np.ascontiguousarray(
            w_out[:, h * DH:(h + 1) * DH].T.astype(ml_dtypes.bfloat16))
        in_maps.append({"xT": xT, "wqkvT": wqkvT, "w_outT": w_outT})
    return in_maps


def kernel(x, w_qkv, w_out):
    from concourse.bass_utils import run_bass_kernel_spmd

    x = np.ascontiguousarray(np.asarray(x), dtype=np.float32)
    w_qkv = np.ascontiguousarray(np.asarray(w_qkv), dtype=np.float32)
    w_out = np.ascontiguousarray(np.asarray(w_out), dtype=np.float32)

    nc = _get_compiled()
    res = run_bass_kernel_spmd(nc, make_in_maps(x, w_qkv, w_out),
                               core_ids=list(range(NCORES)))

    out = np.zeros((C, N), dtype=np.float32)
    for r in res.results:
        out += r["outp"].astype(np.float32)
    return out.reshape(1, C, 16, 16, 16)



# revision 1
# speedup vs baseline: 1.5794x; 1.5794x over previous
"""Multi-head 3D attention (8 heads, C=512, N=16^3=4096) on 8 Trainium2 cores.

Sharding: one head per NeuronCore (head-parallel). Each core receives the
full token activations plus its head's slice of the qkv/out projection
weights, computes its head's attention and its partial contribution to the
output projection; the host sums the 8 fp16 partial outputs in fp32.

Per-core algorithm (S^T orientation -> no transposes anywhere):
  xT   = x.reshape(C, N)                   # [512, 4096] fp16, channel-major
  q/k  = W_{qk} @ xT in ONE matmul group   # [128, 512] psum: q rows 0:64,
                                           #   k rows 64:128 (W_k pre-scaled
                                           #   by A = 1024*log2(e) on host)
  v    = xT.T @ Wv.T                       # [4096, 64] bf16 (keys on parts)
  S^T  = kT-tile.T @ qT                    # 2x [128 keys, 512 q] PSUM = A*s
  P^T  = softmax numerator, column-split across 2 engines per key tile:
           one 512-query half -> ACT:  exp(A*s * 8/A + delta)      (exact)
           other half         -> DVE:  int16(A*s + B) bitcast bf16
                                       (Schraudolph exp, 1 tensor_scalar op)
         halves alternate by kt so every query sees a 50/50 mix; per-tile
         softmax latency ~0.7us, and neither engine exceeds ~60% load
  o_aug= [v, 1].T @ P^T                    # [65, 1024] PSUM; row 64 = denom
  o    = o_aug[:64] * (1/denom)            # reciprocal_approx_fast +
                                           #   gpsimd broadcast + DVE mul
  outp = w_out_h @ o                       # [512, 4096] fp16 partials

Softmax numerics: the Schraudolph bit-trick writes round(A*s + B) as int16
whose bits ARE the bf16 exp(8s+delta): A*s = 128*log2(e)*8s, and
B = 128*(127 + c) + delta*128*log2(e) with c = -0.0427 centering the
piecewise-linear-mantissa error (+-3%) around 1 (HW converts fp32->int16
with round-to-nearest; verified by probe). delta = -3.5 shifts all logits
uniformly (softmax-invariant) to center the observed logit range
[-82.6, 88.1] inside the int16-safe window (-88.0, +88.7); it also pulls
the peak numerator well below fp32-overflow in the o accumulation and the
peak denominator below reciprocal_approx_fast's undefined |x|>~1e38 zone.
Measured end-to-end rel err: 7.5e-3 (gate 2e-2).

Pipeline/schedule (the PE is the bottleneck: S + P@V stream 109 us of
columns at 1 cycle/row fp16, projections ~20 us):
 - x streams in over 3 DMA queues (sync/gpsimd/scalar); q/k/v projections
   are JIT'd inside query group 0 so they hide under the DMA shadow.
 - per key tile the PE issues S_A, S_B then the o-matmuls of kt-3: the
   trail-3 gives the cross-engine softmax chain (sem + 0.7us exp + sem,
   ~1.5us) a ~2.6us window, so the PE never waits on softmax.
 - PSUM: two 3-buf pools of [128,512] S tiles (one per query half, 6
   banks) + single-buffered [65,1024] o psum (2 banks) = 8 banks. The o
   psum is freed ~1.2us after a group ends by a fast unnormalized flush;
   normalization runs in place on SBUF off the critical path (the last
   group instead uses a 512-wide pipelined chain fused into the flush).
 - each group's output projection + fp16 conversion + DMA is interleaved
   into the NEXT group's attention; only qg3's runs as a ~10us tail. Each
   group's first 3 S tiles are pre-emitted in the previous group's tail so
   the o pipeline restarts hot across group boundaries.
 - discretionary PE work is minimized (4 warm-up fillers only): sustained
   PE activity triggers the chip's HAM duty-cycle throttle (k=8 -> k=4 at
   ~75% avg util), which is also the main source of run-to-run variance
   (~195us unthrottled, ~218us when a 50us half-speed window lands
   mid-run). Filler matmuls to fight clock-gating are counterproductive.

Custom-DVE gotcha (HW-verified): InstCustomDveAnt ignores the input AP's
partition offset -- reciprocal_approx_fast on ops[64:65,:] silently read
partition 0. The denominator row is first copied to a partition-0 SBUF
tile with a plain tensor_copy (which handles offsets correctly).
"""

import sys

for _p in ("/opt/trn_rl_repo",):
    if _p not in sys.path:
        sys.path.insert(0, _p)

import math

import numpy as np

C = 512          # channels
N = 4096         # tokens (16*16*16)
HEADS = 8
DH = C // HEADS  # 64
NCORES = 8

KT = 128                 # key-tile size (S^T partition dim)
NKT = N // KT            # 32
QG = 1024                # queries per o-psum accumulation group
NQG = N // QG            # 4
SW = 1024                # S-tile width (queries per exp call)
MV = 512                 # max matmul free dim (one PSUM bank)

A_SCALE = 1024.0 * math.log2(math.e)     # folded into W_k on host
DELTA = -3.5                              # uniform logit shift
C_CORR = -0.0427                          # Schraudolph centering
B_DVE = 128.0 * (127.0 + C_CORR) + DELTA * 128.0 * math.log2(math.e)
EXP_SCALE = 8.0 / A_SCALE

_compiled = None


def _build():
    import concourse.tile as tile
    from concourse import bacc, mybir

    F32 = mybir.dt.float32
    F16 = mybir.dt.float16
    BF16 = mybir.dt.bfloat16
    I16 = mybir.dt.int16
    EXP = mybir.ActivationFunctionType.Exp
    MUL = mybir.AluOpType.mult
    ADD = mybir.AluOpType.add
    NCT = C // 128  # 4 channel tiles

    import os
    kdebug = bool(int(os.environ.get("KDEBUG", "0")))

    nc = bacc.Bacc("TRN2", num_devices=NCORES)
    xT_d = nc.dram_tensor("xT", [C, N], F16, kind="ExternalInput")
    # columns 0:64 = Wq^T, 64:128 = A*Wk^T, 128:192 = Wv^T (this head's rows)
    wqkvT_d = nc.dram_tensor("wqkvT", [C, 3 * DH], F16, kind="ExternalInput")
    # w_out[:, head_cols].T  -> [64, 512]
    w_outT_d = nc.dram_tensor("w_outT", [DH, C], BF16, kind="ExternalInput")
    outp_d = nc.dram_tensor("outp", [C, N], F16, kind="ExternalOutput")
    if kdebug:
        dbg_qT = nc.dram_tensor("dbg_qT", [DH, N], F16, kind="ExternalOutput")
        dbg_kT = nc.dram_tensor("dbg_kT", [DH, N], F16, kind="ExternalOutput")
        dbg_vaug = nc.dram_tensor("dbg_vaug", [128, NKT * (DH + 1)], BF16,
                                  kind="ExternalOutput")
        dbg_osb = nc.dram_tensor("dbg_osb", [DH, N], BF16,
                                 kind="ExternalOutput")
        dbg_recip = nc.dram_tensor("dbg_recip", [1, N], F32,
                                   kind="ExternalOutput")
        dbg_recipb = nc.dram_tensor("dbg_recipb", [DH, N], F32,
                                    kind="ExternalOutput")
        dbg_p = nc.dram_tensor("dbg_p", [128, NKT * SW], BF16,
                               kind="ExternalOutput")

    with tile.TileContext(nc) as tc:
        with tc.tile_pool(name="const", bufs=1) as const:
            # ---- persistent SBUF tensors ----
            xt = [const.tile([128, N], F16, tag=f"x{i}", name=f"x{i}")
                  for i in range(NCT)]
            wqkv = [const.tile([128, 3 * DH], F16, tag=f"w{i}", name=f"w{i}")
                    for i in range(NCT)]
            woutT = const.tile([DH, C], BF16, tag="wo")
            qT = const.tile([DH, N], F16, tag="qT")
            kT = const.tile([DH, N], F16, tag="kT")
            vaug = const.tile([128, NKT, DH + 1], BF16, tag="vaug")
            o_sb = const.tile([DH, N], BF16, tag="o")        # o^T normalized
            den = const.tile([1, N], F32, tag="den")         # softmax denom
            recip = const.tile([1, N], F32, tag="recip")     # 1/denominator
            recipb = const.tile([DH, N], F32, tag="recipb")  # bcast to 64p
            # P^T tiles for one full query group (decouples P@v from exp)
            pstore = const.tile([128, NKT, SW], BF16, tag="pstore")

            # ones column of vaug (o-matmul denominator row), written once
            nc.gpsimd.memset(vaug[:, :, DH:DH + 1], 1.0)
            # per-partition bias AP for the ACT exp (delta logit shift)
            dbias = const.tile([128, 1], F32, tag="dbias")
            nc.vector.memset(dbias, DELTA)

            # inputs across three DMA queues so the ramp-critical pieces land
            # in parallel: slice0 (gates qk0) on sync, wqkv + slice2 on
            # scalar, slice1 on gpsimd; woutT is not needed until qg1
            dma_engs = (nc.sync, nc.gpsimd, nc.scalar, nc.sync)
            for i in range(NCT):
                nc.sync.dma_start(
                    out=wqkv[i], in_=wqkvT_d.ap()[i * 128:(i + 1) * 128, :])
            nc.sync.dma_start(out=woutT, in_=w_outT_d.ap())
            for si, (lo, hi) in enumerate(
                    ((0, 512), (512, 1024), (1024, 2048), (2048, N))):
                for i in range(NCT):
                    dma_engs[si].dma_start(
                        out=xt[i][:, lo:hi],
                        in_=xT_d.ap()[i * 128:(i + 1) * 128, lo:hi])

            def qk_chunk(pool, ch):
                """q AND k projection for token chunk ch in one matmul group:
                psum rows 0:64 = q, 64:128 = A*k (host-folded scale)."""
                sl = slice(ch * MV, (ch + 1) * MV)
                ps = pool.tile([128, MV], F32, tag="s", name=f"psqk{ch}")
                for ct in range(NCT):
                    nc.tensor.matmul(ps, lhsT=wqkv[ct][:, 0:2 * DH],
                                     rhs=xt[ct][:, sl],
                                     start=(ct == 0), stop=(ct == NCT - 1))
                nc.vector.tensor_copy(out=qT[:, sl], in_=ps[0:DH, :])
                nc.vector.tensor_copy(out=kT[:, sl], in_=ps[DH:2 * DH, :])

            def v_tile(pool, kt_i):
                """v projection for key tile kt_i -> vaug[:, kt_i, 0:64]."""
                ps = pool.tile([128, MV], F32, tag="s", name=f"psv{kt_i}")
                for ct in range(NCT):
                    nc.tensor.matmul(ps[:, 0:DH],
                                     lhsT=xt[ct][:, kt_i * KT:(kt_i + 1) * KT],
                                     rhs=wqkv[ct][:, 2 * DH:3 * DH],
                                     start=(ct == 0), stop=(ct == NCT - 1))
                nc.scalar.copy(out=vaug[:, kt_i, 0:DH], in_=ps[:, 0:DH])

            # ---- attention (projections JIT'd into query group 0) ----
            # PSUM: s_ps 6 x [128,512] (6 banks) + ops [65,1024] (2 banks,
            # single-buffered with fast flush) = 8 banks exactly.
            with tc.tile_pool(name="s_psA", bufs=3, space="PSUM") as s_psA, \
                 tc.tile_pool(name="s_psB", bufs=3, space="PSUM") as s_psB, \
                 tc.tile_pool(name="o_ps", bufs=1, space="PSUM") as o_ps, \
                 tc.tile_pool(name="out_sb", bufs=3) as out_sb:
                s_ps = s_psA
                # warm-up fillers need only the FIRST two weight DMAs; the
                # PE enters the projections already at full clock. Kept
                # minimal: PE activity feeds the chip's HAM throttle budget,
                # so every discretionary matmul eventually costs real time.
                last_filler = None
                for wf in range(4):
                    last_filler = s_ps.tile([128, MV], F32, tag="s",
                                            name=f"warm{wf}")
                    nc.tensor.matmul(last_filler[:, 0:192],
                                     lhsT=wqkv[wf % 2][:, 0:128],
                                     rhs=wqkv[(wf + 1) % 2][:, :],
                                     start=True, stop=True,
                                     skip_group_check=True)
                fzt = const.tile([128, 1], F32, tag="fzt")
                nc.vector.tensor_copy(out=fzt, in_=last_filler[:, 0:1])

                def out_block(qgp, ct):
                    """output projection for query group qgp, channel block
                    ct: 2 matmuls -> 2 psum tiles, parallel ACT+DVE fp16
                    copies, one DMA. Interleaved into the NEXT query group's
                    attention so only qg3's blocks run as a tail."""
                    base = qgp * QG
                    pso = []
                    for mv in range(2):
                        p = s_ps.tile([128, MV], F32, tag="s",
                                      name=f"pso{qgp}_{ct}_{mv}")
                        nc.tensor.matmul(
                            p, lhsT=woutT[:, ct * 128:(ct + 1) * 128],
                            rhs=o_sb[:, base + mv * MV:base + (mv + 1) * MV],
                            start=True, stop=True)
                        pso.append(p)
                    ot = out_sb.tile([128, 1024], F16, tag="ot",
                                     name=f"ot{qgp}_{ct}")
                    nc.scalar.copy(out=ot[:, 0:MV], in_=pso[0])
                    nc.vector.tensor_copy(out=ot[:, MV:2 * MV], in_=pso[1])
                    if qgp == 0 and ct == 0:
                        # + 0 * filler keeps the warm-up matmuls alive
                        nc.vector.scalar_tensor_tensor(
                            out=ot[:, 0:1], in0=fzt, scalar=0.0,
                            in1=ot[:, 0:1], op0=MUL, op1=ADD)
                    dma_engs[ct % 3].dma_start(
                        out=outp_d.ap()[ct * 128:(ct + 1) * 128,
                                        base:base + QG],
                        in_=ot)

                # first two q/k chunks feed the first S matmuls; the rest of
                # the projections are emitted just-in-time inside qg 0
                qk_chunk(s_ps, 0)
                qk_chunk(s_ps, 1)

                def emit_s(qgx, ktx):
                    """S matmul pair + column-split softmax dispatch for key
                    tile ktx of query group qgx: one 1-bank psum tile per
                    512-query half so each engine starts as soon as ITS half
                    lands; ACT exps one half, DVE bit-tricks the other,
                    alternating by kt (50/50 exact/approx per query)."""
                    qx0 = qgx * QG
                    sps = [None, None]
                    for mv in range(SW // MV):
                        pool = s_psA if mv == 0 else s_psB
                        sps[mv] = pool.tile([128, MV], F32, tag="s",
                                            name=f"sps{qgx}_{ktx}_{mv}")
                        nc.tensor.matmul(
                            sps[mv],
                            lhsT=kT[:, ktx * KT:(ktx + 1) * KT],
                            rhs=qT[:, qx0 + mv * MV: qx0 + (mv + 1) * MV],
                            start=True, stop=True)
                    a = (ktx + 2 * qgx) % 2
                    ha = slice(a * MV, a * MV + MV)
                    hd = slice((1 - a) * MV, (1 - a) * MV + MV)
                    nc.scalar.activation(out=pstore[:, ktx, ha],
                                         in_=sps[a], func=EXP,
                                         scale=EXP_SCALE, bias=dbias)
                    pi16 = pstore[:, ktx, hd].bitcast(I16)
                    nc.vector.tensor_scalar(out=pi16, in0=sps[1 - a],
                                            scalar1=B_DVE,
                                            scalar2=None, op0=ADD)

                for qg in range(NQG):
                    q0 = qg * QG
                    ops = o_ps.tile([DH + 1, QG], F32, tag="ops",
                                    name=f"ops{qg}")
                    # groups > 0 had their first 3 S tiles pre-emitted in the
                    # previous group's tail, so the o pipeline starts hot
                    s_start = 0 if qg == 0 else 3
                    for kt_i in range(NKT + 3):
                        if qg == 0 and kt_i < NKT:
                            if kt_i % 4 == 0 and kt_i // 4 + 2 < N // MV:
                                qk_chunk(s_ps, kt_i // 4 + 2)
                            v_tile(s_ps, kt_i)
                        if qg >= 1 and kt_i in (6, 12, 18, 24):
                            out_block(qg - 1, (kt_i - 6) // 6)
                        if s_start <= kt_i < NKT:
                            emit_s(qg, kt_i)
                        if kt_i >= 3:
                            # o trails S by THREE tiles: the softmax chain
                            # (sem + exp + sem, ~1.5us) hides well under the
                            # PE work window, so the PE never stalls on it
                            ot_i = kt_i - 3
                            for mv in range(SW // MV):
                                nc.tensor.matmul(
                                    ops[:, mv * MV:(mv + 1) * MV],
                                    lhsT=vaug[:, ot_i, :],
                                    rhs=pstore[:, ot_i, mv * MV:(mv + 1) * MV],
                                    start=(ot_i == 0),
                                    stop=(ot_i == NKT - 1))
                    if qg < NQG - 1:
                        # fast flush (frees the single ops buffer in ~1.2us);
                        # normalization then runs in place on o_sb off-path,
                        # hidden under the next query group
                        sl = slice(q0, q0 + QG)
                        nc.vector.tensor_copy(out=o_sb[:, sl],
                                              in_=ops[0:DH, :])
                        nc.scalar.copy(out=den[:, sl], in_=ops[DH:DH + 1, :])
                        nc.vector.reciprocal_approx_fast(out=recip[:, sl],
                                                         in_=den[:, sl])
                        nc.gpsimd.partition_broadcast(recipb[:, sl],
                                                      recip[:, sl])
                        nc.vector.tensor_mul(o_sb[:, sl], o_sb[:, sl],
                                             recipb[:, sl])
                    else:
                        # last group: nothing hides this chain, so pipeline
                        # it in 512-wide steps across ACT/DVE/POOL with the
                        # normalize fused into the flush (~3.4us vs ~7us)
                        for h2 in range(QG // MV):
                            hps = slice(h2 * MV, (h2 + 1) * MV)
                            hsl = slice(q0 + h2 * MV, q0 + (h2 + 1) * MV)
                            nc.scalar.copy(out=den[:, hsl],
                                           in_=ops[DH:DH + 1, hps])
                            nc.vector.reciprocal_approx_fast(
                                out=recip[:, hsl], in_=den[:, hsl])
                            nc.gpsimd.partition_broadcast(recipb[:, hsl],
                                                          recip[:, hsl])
                            nc.vector.scalar_tensor_tensor(
                                out=o_sb[:, hsl], in0=ops[0:DH, hps],
                                scalar=1.0, in1=recipb[:, hsl],
                                op0=MUL, op1=MUL)

                    if qg + 1 < NQG:
                        # overlap the boundary: next group's first S tiles +
                        # softmax run under this group's trailing o-matmuls
                        for k2 in range(3):
                            emit_s(qg + 1, k2)

                # tail: last query group's output projection
                for ct in range(NCT):
                    out_block(NQG - 1, ct)

            if kdebug:
                nc.sync.dma_start(out=dbg_qT.ap(), in_=qT)
                nc.sync.dma_start(out=dbg_kT.ap(), in_=kT)
                nc.sync.dma_start(out=dbg_vaug.ap(), in_=vaug)
                nc.sync.dma_start(out=dbg_osb.ap(), in_=o_sb)
                nc.sync.dma_start(out=dbg_recip.ap(), in_=recip)
                nc.sync.dma_start(out=dbg_recipb.ap(), in_=recipb)
                nc.sync.dma_start(out=dbg_p.ap(), in_=pstore)

    nc.compile()
    return nc


def _get_compiled():
    global _compiled
    if _compiled is None:
        _compiled = _build()
    return _compiled


def make_in_maps(x, w_qkv, w_out):
    import ml_dtypes
    xT = np.ascontiguousarray(x.reshape(C, N).astype(np.float16))
    in_maps = []
    for h in range(NCORES):
        wq = w_qkv[h * DH:(h + 1) * DH, :]
        wk = w_qkv[C + h * DH:C + (h + 1) * DH, :] * np.float32(A_SCALE)
        wv = w_qkv[2 * C + h * DH:2 * C + (h + 1) * DH, :]
        wqkvT = np.ascontiguousarray(
            np.concatenate([wq, wk, wv], axis=0).T.astype(np.float16))
        w_outT = np.ascontiguousarray(
            w_out[:, h * DH:(h + 1) * DH].T.astype(ml_dtypes.bfloat16))
        in_maps.append({"xT": xT, "wqkvT": wqkvT, "w_outT": w_outT})
    return in_maps


def kernel(x, w_qkv, w_out):
    from concourse.bass_utils import run_bass_kernel_spmd

    x = np.ascontiguousarray(np.asarray(x), dtype=np.float32)
    w_qkv = np.ascontiguousarray(np.asarray(w_qkv), dtype=np.float32)
    w_out = np.ascontiguousarray(np.asarray(w_out), dtype=np.float32)

    nc = _get_compiled()
    res = run_bass_kernel_spmd(nc, make_in_maps(x, w_qkv, w_out),
                               core_ids=list(range(NCORES)))

    out = np.zeros((C, N), dtype=np.float32)
    for r in res.results:
        out += r["outp"].astype(np.float32)
    return out.reshape(1, C, 16, 16, 16)

